# revision 1
# baseline (speedup 1.0000x reference)
"""Trainium2 Bass kernel for a GPT-style transformer block (B=2, T=2048, C=1024, H=16).

Sharding: Megatron-style tensor parallelism over 8 NeuronCores.
  - Attention is head-parallel: each core computes QKV / attention for its 2 heads
    over all 4096 tokens.
  - A small AllToAll (1 MB/core) redistributes attention outputs from
    head-sharded to token-sharded.
  - The output projection, LayerNorm2 and the MLP are token-parallel: each core
    handles its 512-token shard with the full weight matrices.

Everything on-device runs in "transposed" layouts [feature, token] so that
weight matrices act as natural [in, out] stationary operands for the PE array
and per-feature biases broadcast along partitions. LayerNorm affine params are
folded into the weights on the host; the LN1 normalization itself is folded
around the QKV matmul as a per-token affine (raw @ W scaled post-hoc).
Softmax denominators come free from a ones-column appended to V.
"""

from contextlib import ExitStack

import numpy as np
import ml_dtypes

import concourse.bass as bass
import concourse.bacc as bacc
import concourse.mybir as mybir
import concourse.tile as tile
from concourse.bass_utils import run_bass_kernel_spmd

BF16 = mybir.dt.bfloat16
F32 = mybir.dt.float32
AF = mybir.ActivationFunctionType
OP = mybir.AluOpType

N_CORES = 8
B, T, C, H, D = 2, 2048, 1024, 16, 64
NTOK = B * T  # 4096
F = 4 * C  # 4096
LN_EPS = 1e-5
HPC = H // N_CORES  # heads per core = 2
SHARD = NTOK // N_CORES  # 512 tokens per core
NCH = C // 128  # 8 channel blocks
NFB = F // 128  # 32 ffn blocks
NCHUNK = NTOK // 512  # 8 token chunks of 512
SB = 128  # s-block size
VW = D + 1  # V block width incl. ones column = 65

_CACHE = {}



def _pbc(ap, n):
    """Partition-broadcast AP: read `ap` (1-D) n times across partitions."""
    return bass.AP(tensor=ap.tensor, offset=ap.offset,
                   ap=[[0, n]] + [list(x) for x in ap.ap])

_UID = [0]


def _t(pool, shape, dtype, tag):
    _UID[0] += 1
    return pool.tile(shape, dtype, tag=tag, name=f"{tag}_{_UID[0]}")


def _build(with_bias_qkv: bool):
    nc = bacc.Bacc("TRN2", target_bir_lowering=False, debug=False,
                   num_devices=N_CORES)

    # ---- I/O ----
    xT_d = nc.dram_tensor("xT", [C, NTOK], BF16, kind="ExternalInput")
    xTs_d = nc.dram_tensor("xTs", [C, SHARD], F32, kind="ExternalInput")
    wqkv_d = nc.dram_tensor("wqkv", [C, 3 * 128], BF16, kind="ExternalInput")
    wsum_d = nc.dram_tensor("wsum", [1, 3 * 128], BF16, kind="ExternalInput")
    bqkv_d = nc.dram_tensor("bqkv", [1, 3 * 128], BF16, kind="ExternalInput")
    wao_d = nc.dram_tensor("wao", [C, C], BF16, kind="ExternalInput")
    bao_d = nc.dram_tensor("bao", [C], F32, kind="ExternalInput")
    wfc_d = nc.dram_tensor("wfc", [C, F], BF16, kind="ExternalInput")
    bfc_d = nc.dram_tensor("bfc", [F], F32, kind="ExternalInput")
    wmp_d = nc.dram_tensor("wmp", [F, C], BF16, kind="ExternalInput")
    bmp_d = nc.dram_tensor("bmp", [C], F32, kind="ExternalInput")
    mask_d = nc.dram_tensor("mask", [128, 896], BF16, kind="ExternalInput")
    ident_d = nc.dram_tensor("ident", [128, 128], BF16, kind="ExternalInput")
    out_d = nc.dram_tensor("out", [C, SHARD], F32, kind="ExternalOutput")

    with tile.TileContext(nc) as tc, ExitStack() as _es:
            singles = _es.enter_context(tc.tile_pool(name="singles", bufs=1))
            dram = _es.enter_context(tc.tile_pool(name="dram", bufs=1, space="DRAM"))
            psp = _es.enter_context(tc.tile_pool(name="ps", bufs=2, space="PSUM"))
            psyp = _es.enter_context(tc.tile_pool(name="ps_y", bufs=2, space="PSUM"))
            pscp = _es.enter_context(tc.tile_pool(name="ps_c", bufs=2, space="PSUM"))
            # ---------- constants ----------
            eps_t = _t(singles, [128, 1], F32, "eps")
            nc.vector.memset(eps_t[:], LN_EPS)
            ones_t = _t(singles, [128, 1], BF16, "ones")
            nc.vector.memset(ones_t[:], 1.0)
            mask_t = _t(singles, [128, 896], BF16, "mask")
            nc.sync.dma_start(mask_t[:], mask_d[:])
            ident_t = _t(singles, [128, 128], BF16, "ident")
            nc.sync.dma_start(ident_t[:], ident_d[:])
            wsum_t = _t(singles, [1, 384], BF16, "wsum")
            nc.sync.dma_start(wsum_t[:], wsum_d[:])
            bqkv_col_t = _t(singles, [128, 3], F32, "bqkv_col")
            nc.gpsimd.dma_start(
                bqkv_col_t[:],
                bqkv_d.ap()[0, :].rearrange("(o p) -> p o", p=128))
            onesrow_t = _t(singles, [1, 512], BF16, "onesrow")
            nc.vector.memset(onesrow_t[:], 1.0)
            onesrow32_t = _t(singles, [1, 128], F32, "onesrow32")
            nc.vector.memset(onesrow32_t[:], 1.0)
            bao_t = _t(singles, [128, NCH], F32, "bao")
            nc.sync.dma_start(bao_t[:], bao_d.ap().rearrange("(o p) -> p o", p=128))
            bfc_t = _t(singles, [128, NFB], F32, "bfc")
            nc.sync.dma_start(bfc_t[:], bfc_d.ap().rearrange("(o p) -> p o", p=128))
            bmp_t = _t(singles, [128, NCH], F32, "bmp")
            nc.sync.dma_start(bmp_t[:], bmp_d.ap().rearrange("(o p) -> p o", p=128))

            # QKV weights resident: 3 out-blocks of [128ch x 8kb x 128o]
            wqkv_t = []
            for o in range(3):
                wt = _t(singles, [128, NCH, 128], BF16, f"wqkv{o}")
                nc.sync.dma_start(
                    wt[:],
                    wqkv_d.ap()[:, 128 * o:128 * (o + 1)].rearrange(
                        "(kb p) m -> p kb m", p=128),
                )
                wqkv_t.append(wt)

            # attention result tiles (resident, written per chunk)
            qT_t = _t(singles, [128, NTOK], BF16, "qT")
            kT_t = _t(singles, [128, NTOK], BF16, "kT")
            vT_t = _t(singles, [128, NTOK], BF16, "vT")
            qkv_dest = [qT_t, kT_t, vT_t]

            # A2A dram buffers (per shard: 2 heads x (64 y rows + denom row)).
            # Split in two collectives: batch-0 shards (slots 0-3) go in pair 1,
            # batch-1 shards in pair 2; unused slots stay zero so outputs sum.
            a2a_in1 = _t(dram, [N_CORES, HPC, VW, SHARD], BF16, "a2a_in1")
            a2a_out1 = _t(dram, [N_CORES, HPC, VW, SHARD], BF16, "a2a_out1")
            a2a_in2 = _t(dram, [N_CORES, HPC, VW, SHARD], BF16, "a2a_in2")
            a2a_out2 = _t(dram, [N_CORES, HPC, VW, SHARD], BF16, "a2a_out2")
            zt = _t(singles, [128, 520], BF16, "zt")
            nc.vector.memset(zt[:], 0.0)
            for i in range(4, 8):
                nc.sync.dma_start(a2a_in1[i].opt(), zt[:])
            for i in range(0, 4):
                nc.sync.dma_start(a2a_in2[i].opt(), zt[:])

            # ---- Phases A+B+C interleaved: LN1 stats + QKV + attention ----
            with ExitStack() as es1:
                pool_xt = es1.enter_context(tc.tile_pool(name="xt", bufs=8))
                pool_st = es1.enter_context(tc.tile_pool(name="st", bufs=4))
                pool_sqx = es1.enter_context(tc.tile_pool(name="sqx", bufs=4))
                pool_bc = es1.enter_context(tc.tile_pool(name="bc", bufs=8))
                pool_vt = es1.enter_context(tc.tile_pool(name="vt", bufs=2))
                pool_att = es1.enter_context(tc.tile_pool(name="att", bufs=4))
                pool_yt = es1.enter_context(tc.tile_pool(name="yt", bufs=4))

                xt_tiles = {}

                def stats_chunk(g):
                    t0 = 512 * g
                    xt = _t(pool_xt, [128, NCH, 512], BF16, "xt")
                    nc.sync.dma_start(
                        xt[:],
                        xT_d.ap()[:, t0:t0 + 512].rearrange(
                            "(kb p) t -> p kb t", p=128))
                    xt_tiles[g] = xt
                    # LN1 stats: sums of x and x^2 over channels via PE
                    s1p = _t(psp, [1, 512], F32, "ps")
                    s2p = _t(psp, [1, 512], F32, "ps")
                    for kb in range(NCH):
                        nc.tensor.matmul(s1p[:], ones_t[:], xt[:, kb, :],
                                         start=(kb == 0), stop=(kb == NCH - 1))
                    for kb in range(NCH):
                        sq = _t(pool_sqx, [128, 512], BF16, "sqx")
                        if kb % 2 == 0:
                            nc.scalar.activation(sq[:], xt[:, kb, :], AF.Square)
                        else:
                            nc.vector.tensor_tensor(sq[:], xt[:, kb, :],
                                                    xt[:, kb, :], OP.mult)
                        nc.tensor.matmul(s2p[:], ones_t[:], sq[:],
                                         start=(kb == 0), stop=(kb == NCH - 1))
                    nmu = _t(pool_bc, [1, 512], BF16, "nmu")
                    nc.vector.tensor_scalar(out=nmu[:], in0=s1p[:],
                                            scalar1=-1.0 / C, scalar2=0.0,
                                            op0=OP.mult, op1=OP.add)
                    mur = _t(pool_st, [1, 512], F32, "mur")
                    nc.vector.tensor_scalar(out=mur[:], in0=s1p[:],
                                            scalar1=1.0 / C, scalar2=0.0,
                                            op0=OP.mult, op1=OP.add)
                    msq = _t(pool_st, [1, 512], F32, "msq")
                    nc.vector.tensor_tensor(msq[:], mur[:], mur[:], OP.mult)
                    var = _t(pool_st, [1, 512], F32, "var")
                    nc.vector.scalar_tensor_tensor(
                        out=var[:], in0=s2p[:], scalar=1.0 / C, in1=msq[:],
                        op0=OP.mult, op1=OP.subtract)
                    srt = _t(pool_st, [1, 512], F32, "srt")
                    nc.scalar.activation(srt[:], var[:], AF.Sqrt,
                                         bias=eps_t[0:1, :], scale=1.0)
                    arow = _t(pool_st, [1, 512], F32, "arow")
                    nc.vector.reciprocal_approx_fast(arow[:], srt[:])
                    abp = _t(psp, [128, 512], F32, "ps")
                    nc.tensor.matmul(abp[:], onesrow32_t[:], arow[:])
                    abc = _t(pool_bc, [128, 512], F32, "abc")
                    nc.vector.tensor_copy(abc[:], abp[:])
                    return abc, nmu

                ab_tiles = {}

                def qkv_chunk(g):
                    t0 = 512 * g
                    xt = xt_tiles.pop(g)
                    abc, nmu = ab_tiles.pop(g)
                    for o in range(3):
                        ps = _t(psp, [128, 512], F32, "ps")
                        for kb in range(NCH):
                            nc.tensor.matmul(ps[:], wqkv_t[o][:, kb, :],
                                             xt[:, kb, :],
                                             start=(kb == 0), stop=False)
                        # rank-1 terms: wsum (x) b  [+ bias (x) ones]
                        nc.tensor.matmul(ps[:], wsum_t[:, 128 * o:128 * (o + 1)],
                                         nmu[:], start=False,
                                         stop=True)
                        dst = qkv_dest[o][:, t0:t0 + 512]
                        nc.vector.tensor_tensor(dst, ps[:], abc[:], OP.mult)
                        if with_bias_qkv:
                            nc.vector.tensor_scalar(
                                out=dst, in0=dst,
                                scalar1=bqkv_col_t[:, o:o + 1], scalar2=0.0,
                                op0=OP.add, op1=OP.add)

                vt = {}

                def att_chunk(b, j):
                    g = 4 * b + j
                    t0 = 512 * g
                    nblk = 4 * j + 4
                    if j == 0:
                        for h in range(HPC):
                            v = _t(pool_vt, [128, T // SB, VW], BF16, f"vt{h}")
                            nc.vector.memset(v[:, :, D:VW], 1.0)
                            vt[b, h] = v
                    # transpose V for the newly available s-blocks
                    for i in range(4 * j, 4 * j + 4):
                        s0 = 2048 * b + SB * i
                        for h in range(HPC):
                            pst = _t(psp, [128, D], BF16, "ps")
                            nc.tensor.transpose(
                                pst[:],
                                vT_t[64 * h:64 * (h + 1), s0:s0 + SB],
                                ident_t[64 * h:64 * (h + 1),
                                        64 * h:64 * (h + 1)],
                            )
                            nc.vector.tensor_copy(vt[b, h][:, i, 0:D], pst[:])
                    psy = [_t(psyp, [VW, 512], F32, "psy") for h in range(HPC)]
                    for i in range(nblk):
                        s0 = 2048 * b + SB * i
                        m = i - 4 * j  # >= 0 on diagonal blocks
                        f0 = 128 * m if m >= 0 else 0  # causal: t-f0 cols only
                        psc = _t(pscp, [128, HPC, 512], F32, "psc")
                        for h in range(HPC):
                            nc.tensor.matmul(
                                psc[:, h, 0:512 - f0],
                                kT_t[64 * h:64 * (h + 1), s0:s0 + SB],
                                qT_t[64 * h:64 * (h + 1), t0 + f0:t0 + 512],
                                tile_position=(64 * h, 0),
                            )
                        at = _t(pool_att, [128, HPC, 512], BF16, "att")
                        nc.scalar.activation(at[:, :, 0:512 - f0],
                                             psc[:, :, 0:512 - f0], AF.Exp)
                        if m >= 0:  # diagonal: mask boundary block only
                            for h in range(HPC):
                                nc.vector.tensor_tensor(
                                    at[:, h, 0:128], at[:, h, 0:128],
                                    mask_t[:, 384:512], OP.mult)
                        for h in range(HPC):
                            nc.tensor.matmul(
                                psy[h][:, f0:512], vt[b, h][:, i, :],
                                at[:, h, 0:512 - f0],
                                start=(i == 0), stop=(i == nblk - 1))
                    # ship raw y + softmax denominator row; normalize post-A2A
                    a2a_in = a2a_in1 if b == 0 else a2a_in2
                    for h in range(HPC):
                        yt = _t(pool_yt, [VW, 512], BF16, "yt")
                        nc.vector.tensor_copy(yt[:], psy[h][:])
                        nc.sync.dma_start(a2a_in[g, h], yt[:])

                for g in range(NCHUNK):
                    ab_tiles[g] = stats_chunk(g)
                for b in range(B):
                    for j in range(4):
                        qkv_chunk(4 * b + j)
                        att_chunk(b, j)
                    # AllToAll for this batch's shards overlaps the next batch
                    nc.gpsimd.collective_compute(
                        "AllToAll", OP.bypass,
                        replica_groups=[list(range(N_CORES))],
                        ins=[(a2a_in1 if b == 0 else a2a_in2).opt()],
                        outs=[(a2a_out1 if b == 0 else a2a_out2).opt()],
                    )

            # ---------- Phase D: AO proj + LN2 + MLP on the token shard ----
            with ExitStack() as es3:
                pool_x3 = es3.enter_context(tc.tile_pool(name="x3", bufs=1))
                pool_ya = es3.enter_context(tc.tile_pool(name="ya", bufs=2))
                pool_yn = es3.enter_context(tc.tile_pool(name="yn", bufs=8))
                pool_ao = es3.enter_context(tc.tile_pool(name="ao", bufs=8))
                pool_aob = es3.enter_context(tc.tile_pool(name="aob", bufs=8))
                pool_sq = es3.enter_context(tc.tile_pool(name="sq", bufs=2))
                pool_h2 = es3.enter_context(tc.tile_pool(name="h2", bufs=8))
                pool_mt = es3.enter_context(tc.tile_pool(name="mt", bufs=NFB))
                pool_w3 = es3.enter_context(tc.tile_pool(name="w3", bufs=3))
                pool_wm = es3.enter_context(tc.tile_pool(name="wm", bufs=2))
                pool_row2 = es3.enter_context(tc.tile_pool(name="row2", bufs=1))
                pool_bc2 = es3.enter_context(tc.tile_pool(name="bc2", bufs=1))
                pool_tmp2 = es3.enter_context(tc.tile_pool(name="tmp2", bufs=2))
                pool_ot = es3.enter_context(tc.tile_pool(name="ot", bufs=2))
                xts = _t(pool_x3, [128, NCH, 512], F32, "xts")
                nc.sync.dma_start(
                    xts[:], xTs_d.ap().rearrange("(kb p) t -> p kb t", p=128))
                yall = []
                for i in range(N_CORES):
                    yr1 = _t(pool_ya, [128, 512], BF16, "yr")
                    yr2 = _t(pool_ya, [128, 512], BF16, "yr")
                    dn1 = _t(pool_ya, [128, 512], F32, "dn")
                    dn2 = _t(pool_ya, [128, 512], F32, "dn")
                    for h in range(HPC):
                        nc.sync.dma_start(yr1[64 * h:64 * (h + 1), :],
                                          a2a_out1[i, h, 0:D, :])
                        nc.sync.dma_start(yr2[64 * h:64 * (h + 1), :],
                                          a2a_out2[i, h, 0:D, :])
                        nc.gpsimd.dma_start(
                            dn1[64 * h:64 * (h + 1), :],
                            _pbc(a2a_out1[i, h, D, :], D))
                        nc.gpsimd.dma_start(
                            dn2[64 * h:64 * (h + 1), :],
                            _pbc(a2a_out2[i, h, D, :], D))
                    yr = _t(pool_ya, [128, 512], BF16, "yrs")
                    nc.vector.tensor_tensor(yr[:], yr1[:], yr2[:], OP.add)
                    dn = _t(pool_ya, [128, 512], F32, "dns")
                    nc.vector.tensor_tensor(dn[:], dn1[:], dn2[:], OP.add)
                    dr = _t(pool_ya, [128, 512], F32, "dr")
                    nc.vector.reciprocal_approx_fast(dr[:], dn[:])
                    ya = _t(pool_yn, [128, 512], BF16, "ya")
                    nc.vector.tensor_tensor(ya[:], yr[:], dr[:], OP.mult)
                    yall.append(ya)
                aot, aob = [], []
                for w in range(NCH):
                    wt = _t(pool_w3, [128, NCH, 128], BF16, "w3")
                    nc.sync.dma_start(
                        wt[:],
                        wao_d.ap()[:, 128 * w:128 * (w + 1)].rearrange(
                            "(kb p) m -> p kb m", p=128))
                    ps = _t(psp, [128, 512], F32, "ps")
                    for i in range(NCH):
                        nc.tensor.matmul(ps[:], wt[:, i, :], yall[i][:],
                                         start=(i == 0), stop=(i == NCH - 1))
                    ao = _t(pool_ao, [128, 512], F32, "ao")
                    nc.vector.scalar_tensor_tensor(
                        out=ao[:], in0=ps[:], scalar=bao_t[:, w:w + 1],
                        in1=xts[:, w, :], op0=OP.add, op1=OP.add)
                    ab = _t(pool_aob, [128, 512], BF16, "aob")
                    nc.vector.tensor_copy(ab[:], ao[:])
                    aot.append(ao)
                    aob.append(ab)
                # LN2 stats over channels via ones-matmuls
                ps1 = _t(psyp, [1, 512], F32, "psy")
                for w in range(NCH):
                    nc.tensor.matmul(ps1[:], ones_t[:], aob[w][:],
                                     start=(w == 0), stop=(w == NCH - 1))
                ps2 = _t(psyp, [1, 512], F32, "psy")
                for w in range(NCH):
                    sq = _t(pool_sq, [128, 512], BF16, "sq")
                    nc.vector.tensor_tensor(sq[:], aob[w][:], aob[w][:], OP.mult)
                    nc.tensor.matmul(ps2[:], ones_t[:], sq[:],
                                     start=(w == 0), stop=(w == NCH - 1))
                mur = _t(pool_row2, [1, 512], F32, "mur")
                nc.vector.tensor_scalar(out=mur[:], in0=ps1[:], scalar1=1.0 / C,
                                        scalar2=0.0, op0=OP.mult, op1=OP.add)
                e2r = _t(pool_row2, [1, 512], F32, "e2r")
                nc.vector.tensor_scalar(out=e2r[:], in0=ps2[:], scalar1=1.0 / C,
                                        scalar2=0.0, op0=OP.mult, op1=OP.add)
                msq = _t(pool_row2, [1, 512], F32, "msq")
                nc.vector.tensor_tensor(msq[:], mur[:], mur[:], OP.mult)
                varr = _t(pool_row2, [1, 512], F32, "varr")
                nc.vector.tensor_tensor(varr[:], e2r[:], msq[:], OP.subtract)
                srow2 = _t(pool_row2, [1, 512], F32, "srow2")
                nc.scalar.activation(srow2[:], varr[:], AF.Sqrt,
                                     bias=eps_t[0:1, :], scale=1.0)
                rrow2 = _t(pool_row2, [1, 512], F32, "rrow2")
                nc.vector.reciprocal_approx_fast(rrow2[:], srow2[:])
                mup = _t(psp, [128, 512], F32, "ps")
                nc.tensor.matmul(mup[:], onesrow32_t[:], mur[:])
                mubc = _t(pool_bc2, [128, 512], F32, "mubc")
                nc.vector.tensor_copy(mubc[:], mup[:])
                rbp = _t(psp, [128, 512], F32, "ps")
                nc.tensor.matmul(rbp[:], onesrow32_t[:], rrow2[:])
                rbc2 = _t(pool_bc2, [128, 512], F32, "rbc2")
                nc.vector.tensor_copy(rbc2[:], rbp[:])
                h2 = []
                for w in range(NCH):
                    tp = _t(pool_tmp2, [128, 512], F32, "tmp2")
                    nc.vector.tensor_tensor(tp[:], aot[w][:], mubc[:], OP.subtract)
                    ht = _t(pool_h2, [128, 512], BF16, "h2")
                    nc.vector.tensor_tensor(ht[:], tp[:], rbc2[:], OP.mult)
                    h2.append(ht)
                # FC + GELU
                mt = []
                for fg in range(NFB // 4):
                    wt = _t(pool_w3, [128, NCH, 4, 128], BF16, "w3")
                    nc.sync.dma_start(
                        wt[:],
                        wfc_d.ap()[:, 512 * fg:512 * (fg + 1)].rearrange(
                            "(kb p) (fs m) -> p kb fs m", p=128, m=128))
                    for fs in range(4):
                        f = 4 * fg + fs
                        ps = _t(psp, [128, 512], F32, "ps")
                        for cb in range(NCH):
                            nc.tensor.matmul(ps[:], wt[:, cb, fs, :], h2[cb][:],
                                             start=(cb == 0),
                                             stop=(cb == NCH - 1))
                        m = _t(pool_mt, [128, 512], BF16, "mt")
                        nc.scalar.activation(m[:], ps[:], AF.Gelu,
                                             bias=bfc_t[:, f:f + 1], scale=1.0)
                        mt.append(m)
                # MP + bias + residual -> out
                for w in range(NCH):
                    wt = _t(pool_wm, [128, NFB, 128], BF16, "wm")
                    nc.sync.dma_start(
                        wt[:],
                        wmp_d.ap().rearrange("(fb p) o -> p fb o", p=128)[
                            :, :, 128 * w:128 * (w + 1)])
                    ps = _t(psp, [128, 512], F32, "ps")
                    for f in range(NFB):
                        nc.tensor.matmul(ps[:], wt[:, f, :], mt[f][:],
                                         start=(f == 0), stop=(f == NFB - 1))
                    ot = _t(pool_ot, [128, 512], F32, "ot")
                    nc.vector.scalar_tensor_tensor(
                        out=ot[:], in0=ps[:], scalar=bmp_t[:, w:w + 1],
                        in1=aot[w][:], op0=OP.add, op1=OP.add)
                    nc.sync.dma_start(out_d[128 * w:128 * (w + 1), :], ot[:])

    nc.compile()
    return nc


def _prep(inputs):
    """Host-side preprocessing: fold LN affines into weights, slice per core."""
    f32 = np.float32
    bf16 = ml_dtypes.bfloat16
    x = np.asarray(inputs["x"], f32).reshape(NTOK, C)
    W_qkv = np.asarray(inputs["W_qkv"], f32)
    b_qkv = np.asarray(inputs["b_qkv"], f32)
    W_ao = np.asarray(inputs["W_ao"], f32)
    b_ao = np.asarray(inputs["b_ao"], f32)
    W_fc = np.asarray(inputs["W_fc"], f32)
    b_fc = np.asarray(inputs["b_fc"], f32)
    W_mp = np.asarray(inputs["W_mp"], f32)
    b_mp = np.asarray(inputs["b_mp"], f32)
    g1 = np.asarray(inputs["g1"], f32)
    be1 = np.asarray(inputs["be1"], f32)
    g2 = np.asarray(inputs["g2"], f32)
    be2 = np.asarray(inputs["be2"], f32)

    Wq_eff = W_qkv * g1[:, None]
    bq_eff = b_qkv + be1 @ W_qkv
    # fold 1/sqrt(D) into the Q columns
    Wq_eff[:, :C] *= 1.0 / np.sqrt(D)
    bq_eff[:C] *= 1.0 / np.sqrt(D)
    Wfc_eff = W_fc * g2[:, None]
    bfc_eff = b_fc + be2 @ W_fc

    xT = np.ascontiguousarray(x.T)
    xT_bf = xT.astype(bf16)
    mask = (np.arange(128)[:, None] <= (np.arange(896)[None, :] - 384)).astype(bf16)
    ident = np.eye(128, dtype=bf16)

    wao_bf = W_ao.astype(bf16)
    wfc_bf = Wfc_eff.astype(bf16)
    wmp_bf = W_mp.astype(bf16)

    with_bias_qkv = bool(np.any(bq_eff != 0.0))

    in_maps = []
    for r in range(N_CORES):
        cs = 128 * r
        wq_core = np.concatenate(
            [Wq_eff[:, cs:cs + 128], Wq_eff[:, C + cs:C + cs + 128],
             Wq_eff[:, 2 * C + cs:2 * C + cs + 128]], axis=1)
        bq_core = np.concatenate(
            [bq_eff[cs:cs + 128], bq_eff[C + cs:C + cs + 128],
             bq_eff[2 * C + cs:2 * C + cs + 128]])
        wsum_core = wq_core.sum(axis=0).astype(f32)
        in_maps.append({
            "xT": xT_bf,
            "xTs": np.ascontiguousarray(xT[:, SHARD * r:SHARD * (r + 1)]),
            "wqkv": wq_core.astype(bf16),
            "wsum": np.ascontiguousarray(wsum_core).astype(bf16).reshape(1, -1),
            "bqkv": np.ascontiguousarray(bq_core).astype(bf16).reshape(1, -1),
            "wao": wao_bf,
            "bao": b_ao,
            "wfc": wfc_bf,
            "bfc": bfc_eff.astype(f32),
            "wmp": wmp_bf,
            "bmp": b_mp,
            "mask": mask,
            "ident": ident,
        })
    return in_maps, with_bias_qkv


def kernel(_trace=False, _trace_kwargs=None, **inputs):
    in_maps, with_bias_qkv = _prep(inputs)
    key = ("nc", with_bias_qkv)
    if key not in _CACHE:
        _CACHE[key] = _build(with_bias_qkv)
    nc = _CACHE[key]
    res = run_bass_kernel_spmd(
        nc, in_maps, core_ids=list(range(N_CORES)),
        trace=_trace, **(_trace_kwargs or {}))
    _CACHE["last_results"] = res
    out = np.concatenate(
        [np.asarray(res.results[r]["out"]).T for r in range(N_CORES)], axis=0)
    return np.ascontiguousarray(out.reshape(B, T, C)).astype(np.float32)



# revision 16
# speedup vs baseline: 1.1438x; 1.1438x over previous
"""Trainium2 Bass kernel for a GPT-style transformer block (B=2, T=2048, C=1024, H=16).

Sharding: Megatron-style tensor parallelism over 8 NeuronCores.
  - Attention is head-parallel: each core computes QKV / attention for its 2 heads
    over all 4096 tokens.
  - Per batch, a small AllToAll (512 KB/core) redistributes softmax-normalized
    attention outputs from head-sharded to token-sharded.
  - The output projection, LayerNorm2 and the MLP are token-parallel on
    256-token half-shards: each core owns 256 tokens of batch 0 plus 256 of
    batch 1, so the batch-0 half of the MLP runs while the batch-1 AllToAll is
    still in flight.

Layouts are "transposed" [feature, token] so weight matrices are natural
stationary operands and per-feature biases broadcast along partitions. LN
affines are folded into weights on the host; LN1 normalization is folded
around the QKV matmul as a per-token affine. rstd comes from
exp(-0.5*ln(var+eps)) so ScalarE stays inside the ln/exp table set that
softmax needs. The LN1 per-token scale is broadcast across partitions with a GpSimd
stride-0 DMA; softmax denominators are replicated across 64 PSUM rows for
free by ones-columns in the V stationary; V tiles are transposed with the
DMA xbar on the Scalar queue; collectives run on the GpSimd queue.
"""

from contextlib import ExitStack

import numpy as np
import ml_dtypes

import concourse.bass as bass
import concourse.bacc as bacc
import concourse.mybir as mybir
import concourse.tile as tile
from concourse.bass_utils import run_bass_kernel_spmd

BF16 = mybir.dt.bfloat16
F32 = mybir.dt.float32
AF = mybir.ActivationFunctionType
OP = mybir.AluOpType

N_CORES = 8
B, T, C, H, D = 2, 2048, 1024, 16, 64
NTOK = B * T  # 4096
F = 4 * C  # 4096
LN_EPS = 1e-5
HPC = H // N_CORES  # heads per core = 2
HALF = 256  # phase-D half-shard tokens (per batch)
NCH = C // 128  # 8 channel blocks
NFB = F // 128  # 32 ffn blocks
NCHUNK = NTOK // 512  # 8 token chunks of 512
SB = 128  # s-block size
VW = D + 1  # V block width incl. ones column = 65

_CACHE = {}

_UID = [0]


def _t(pool, shape, dtype, tag):
    _UID[0] += 1
    return pool.tile(shape, dtype, tag=tag, name=f"{tag}_{_UID[0]}")


def _pbc(ap, n):
    """Partition-broadcast AP: read `ap` n times across partitions."""
    return bass.AP(tensor=ap.tensor, offset=ap.offset,
                   ap=[[0, n]] + [list(x) for x in ap.ap])


def _build(with_bias_qkv: bool):
    nc = bacc.Bacc("TRN2", target_bir_lowering=False, debug=False,
                   num_devices=N_CORES)

    # ---- I/O ----
    xT_d = nc.dram_tensor("xT", [C, NTOK], BF16, kind="ExternalInput")
    xTs_d = nc.dram_tensor("xTs", [C, 2 * HALF], F32, kind="ExternalInput")
    wqkv_d = nc.dram_tensor("wqkv", [C, 3 * 128], BF16, kind="ExternalInput")
    wsum_d = nc.dram_tensor("wsum", [1, 3 * 128], BF16, kind="ExternalInput")
    bqkv_d = nc.dram_tensor("bqkv", [1, 3 * 128], BF16, kind="ExternalInput")
    wao_d = nc.dram_tensor("wao", [C, C], BF16, kind="ExternalInput")
    bao_d = nc.dram_tensor("bao", [C], F32, kind="ExternalInput")
    wfc_d = nc.dram_tensor("wfc", [C, F], BF16, kind="ExternalInput")
    bfc_d = nc.dram_tensor("bfc", [F], F32, kind="ExternalInput")
    wmp_d = nc.dram_tensor("wmp", [F, C], BF16, kind="ExternalInput")
    bmp_d = nc.dram_tensor("bmp", [C], F32, kind="ExternalInput")
    mask_d = nc.dram_tensor("mask", [128, 128], BF16, kind="ExternalInput")
    ident_d = nc.dram_tensor("ident", [128, 128], BF16, kind="ExternalInput")
    out_d = nc.dram_tensor("out", [C, 2 * HALF], F32, kind="ExternalOutput")

    with tile.TileContext(nc) as tc, ExitStack() as _es:
            singles = _es.enter_context(tc.tile_pool(name="singles", bufs=1))
            dram = _es.enter_context(tc.tile_pool(name="dram", bufs=1, space="DRAM"))

            with ExitStack() as es1:
                # PSUM: qkv+stats 2 banks, psy 2 banks, psc 4 banks = 8
                psq = es1.enter_context(tc.tile_pool(name="psq", bufs=2,
                                                     space="PSUM"))
                psyp = es1.enter_context(tc.tile_pool(name="ps_y", bufs=2,
                                                      space="PSUM"))
                pscp = es1.enter_context(tc.tile_pool(name="ps_c", bufs=2,
                                                      space="PSUM"))
                pool_xt = es1.enter_context(tc.tile_pool(name="xt", bufs=NCHUNK))
                pool_sqx = es1.enter_context(tc.tile_pool(name="sqx", bufs=17))
                pool_row = es1.enter_context(tc.tile_pool(name="row", bufs=2))
                pool_bc = es1.enter_context(tc.tile_pool(name="bc", bufs=3))
                pool_vt = es1.enter_context(tc.tile_pool(name="vt", bufs=2))
                pool_att = es1.enter_context(tc.tile_pool(name="att", bufs=4))
                pool_yt = es1.enter_context(tc.tile_pool(name="yt", bufs=4))

                xt_tiles = {}

                def load_chunk(g):
                    t0 = 512 * g
                    xt = _t(pool_xt, [128, NCH, 512], BF16, "xt")
                    nc.sync.dma_start(
                        xt[:],
                        xT_d.ap()[:, t0:t0 + 512].rearrange(
                            "(kb p) t -> p kb t", p=128))
                    xt_tiles[g] = xt

                # chunk 0 first so PE can start ASAP, then weights, then rest
                load_chunk(0)

                ones_t = _t(singles, [128, 1], BF16, "ones")
                nc.vector.memset(ones_t[:], 1.0)
                eps_t = _t(singles, [128, 1], F32, "eps")
                nc.vector.memset(eps_t[:], LN_EPS)
                onesrow32_t = _t(singles, [1, 128], F32, "onesrow32")
                nc.vector.memset(onesrow32_t[:], 1.0)

                wqkv_t = []
                for o in range(3):
                    wt = _t(singles, [128, NCH, 128], BF16, f"wqkv{o}")
                    nc.sync.dma_start(
                        wt[:],
                        wqkv_d.ap()[:, 128 * o:128 * (o + 1)].rearrange(
                            "(kb p) m -> p kb m", p=128),
                    )
                    wqkv_t.append(wt)
                wsum_t = _t(singles, [1, 384], BF16, "wsum")
                nc.sync.dma_start(wsum_t[:], wsum_d[:])

                for g in range(1, NCHUNK):
                    load_chunk(g)

                mask_t = _t(singles, [128, 128], BF16, "mask")
                nc.sync.dma_start(mask_t[:], mask_d[:])
                ident_t = _t(singles, [128, 128], BF16, "ident")
                nc.sync.dma_start(ident_t[:], ident_d[:])
                bqkv_col_t = _t(singles, [128, 3], F32, "bqkv_col")
                if with_bias_qkv:
                    nc.gpsimd.dma_start(
                        bqkv_col_t[:],
                        bqkv_d.ap()[0, :].rearrange("(o p) -> p o", p=128))

                # attention operand tiles (resident, written per chunk)
                qT_t = _t(singles, [128, NTOK], BF16, "qT")
                kT_t = _t(singles, [128, NTOK], BF16, "kT")
                vT_t = _t(singles, [128, NTOK], BF16, "vT")
                qkv_dest = [qT_t, kT_t, vT_t]

                # DRAM bounce rows for the abc broadcast (stride-0 DMA
                # needs a DRAM source)
                arow_d = _t(dram, [NCHUNK, 512], F32, "arow_d")
                den_d = _t(dram, [NCHUNK, HPC, 512], F32, "den_d")
                # A2A dram buffers: [dest_core, head, d, 256] per batch
                a2a_in1 = _t(dram, [N_CORES, HPC, D, HALF], BF16, "a2a_in1")
                a2a_out1 = _t(dram, [N_CORES, HPC, D, HALF], BF16, "a2a_out1")
                a2a_in2 = _t(dram, [N_CORES, HPC, D, HALF], BF16, "a2a_in2")
                a2a_out2 = _t(dram, [N_CORES, HPC, D, HALF], BF16, "a2a_out2")

                sq_tiles = {}

                def squares(g):
                    xt = xt_tiles[g]
                    sq = []
                    n_act = 6 if g < 2 else 2
                    for kb in range(NCH):
                        s = _t(pool_sqx, [128, 512], BF16, "sqx")
                        if kb < n_act:
                            nc.scalar.activation(s[:], xt[:, kb, :], AF.Square)
                        else:
                            nc.vector.tensor_tensor(s[:], xt[:, kb, :],
                                                    xt[:, kb, :], OP.mult)
                        sq.append(s)
                    sq_tiles[g] = sq

                ab_tiles = {}

                def stats_chunk(g):
                    """s1 = sum_c x (row 0), s2 = sum_c x^2 (row 32) in one
                    PSUM bank; then the row chain and the abc broadcast."""
                    xt = xt_tiles[g]
                    sq = sq_tiles.pop(g)
                    sp = _t(psq, [64, 512], F32, "psq")
                    for kb in range(NCH):
                        nc.tensor.matmul(sp[0:1, :], ones_t[:], xt[:, kb, :],
                                         start=(kb == 0), stop=(kb == NCH - 1))
                    for kb in range(NCH):
                        # bank already cleared by the s1 chain's start
                        nc.tensor.matmul(sp[32:33, :], ones_t[:], sq[kb][:],
                                         start=False, stop=(kb == NCH - 1),
                                         skip_group_check=True)
                    nmu = _t(pool_row, [1, 512], BF16, "nmu")
                    nc.vector.tensor_scalar(out=nmu[:], in0=sp[0:1, :],
                                            scalar1=-1.0 / C, scalar2=0.0,
                                            op0=OP.mult, op1=OP.add)
                    s1sq = _t(pool_row, [1, 512], F32, "s1sq")
                    nc.scalar.activation(s1sq[:], sp[0:1, :], AF.Square)
                    # varp = s2 - s1^2/C  (= var*C)
                    varp = _t(pool_row, [1, 512], F32, "varp")
                    nc.vector.scalar_tensor_tensor(
                        out=varp[:], in0=s1sq[:], scalar=-1.0 / C,
                        in1=sp[32:33, :], op0=OP.mult, op1=OP.add)
                    # rstd = exp(-0.5*ln(varp/C + eps)) -- stays in ln/exp set
                    lnv = _t(pool_row, [1, 512], F32, "lnv")
                    nc.scalar.activation(lnv[:], varp[:], AF.Ln,
                                         bias=eps_t[0:1, :], scale=1.0 / C)
                    arow = _t(pool_row, [1, 512], F32, "arow")
                    nc.scalar.activation(arow[:], lnv[:], AF.Exp, scale=-0.5)
                    nc.sync.dma_start(arow_d[g], arow[0:1, :])
                    abc = _t(pool_bc, [128, 512], F32, "abc")
                    nc.gpsimd.dma_start(abc[:], _pbc(arow_d[g], 128))
                    ab_tiles[g] = (abc, nmu)

                def qkv_chunk(g):
                    t0 = 512 * g
                    xt = xt_tiles.pop(g)
                    abc, nmu = ab_tiles.pop(g)
                    for o in range(3):
                        ps = _t(psq, [128, 512], F32, "psq")
                        for kb in range(NCH):
                            nc.tensor.matmul(ps[:], wqkv_t[o][:, kb, :],
                                             xt[:, kb, :],
                                             start=(kb == 0), stop=False)
                        # rank-1 term: wsum (x) nmu subtracts the mean
                        nc.tensor.matmul(ps[:], wsum_t[:, 128 * o:128 * (o + 1)],
                                         nmu[:], start=False, stop=True)
                        dst = qkv_dest[o][:, t0:t0 + 512]
                        nc.vector.tensor_tensor(dst, ps[:], abc[:], OP.mult)
                        if with_bias_qkv:
                            nc.vector.tensor_scalar(
                                out=dst, in0=dst,
                                scalar1=bqkv_col_t[:, o:o + 1], scalar2=0.0,
                                op0=OP.add, op1=OP.add)

                vt = {}

                def att_chunk(b, j):
                    g = 4 * b + j
                    t0 = 512 * g
                    nblk = 4 * j + 4
                    if j == 0:
                        for h in range(HPC):
                            v = _t(pool_vt, [128, T // SB, VW], BF16, f"vt{h}")
                            nc.vector.memset(v[:, :, D:VW], 1.0)
                            vt[b, h] = v
                    # transpose V for the newly available s-blocks
                    for i in range(4 * j, 4 * j + 4):
                        s0 = 2048 * b + SB * i
                        for h in range(HPC):
                            pst = _t(psq, [128, D], BF16, "psq")
                            nc.tensor.transpose(
                                pst[:],
                                vT_t[64 * h:64 * (h + 1), s0:s0 + SB],
                                ident_t[64 * h:64 * (h + 1),
                                        64 * h:64 * (h + 1)])
                            nc.vector.tensor_copy(vt[b, h][:, i, 0:D], pst[:])
                    psy = [_t(psyp, [VW, 512], F32, "psy") for h in range(HPC)]
                    for i in range(nblk):
                        s0 = 2048 * b + SB * i
                        m = i - 4 * j  # >= 0 on diagonal blocks
                        f0 = 128 * m if m >= 0 else 0  # causal: t-f0 cols only
                        psc = _t(pscp, [128, HPC, 512], F32, "psc")
                        for h in range(HPC):
                            nc.tensor.matmul(
                                psc[:, h, 0:512 - f0],
                                kT_t[64 * h:64 * (h + 1), s0:s0 + SB],
                                qT_t[64 * h:64 * (h + 1), t0 + f0:t0 + 512],
                                tile_position=(64 * h, 0),
                            )
                        at = _t(pool_att, [128, HPC, 512], BF16, "att")
                        nc.scalar.activation(at[:, :, 0:512 - f0],
                                             psc[:, :, 0:512 - f0], AF.Exp)
                        if m >= 0:  # diagonal: mask boundary block only
                            for h in range(HPC):
                                nc.vector.tensor_tensor(
                                    at[:, h, 0:128], at[:, h, 0:128],
                                    mask_t[:], OP.mult)
                        for h in range(HPC):
                            nc.tensor.matmul(
                                psy[h][:, f0:512], vt[b, h][:, i, :],
                                at[:, h, 0:512 - f0],
                                start=(i == 0), stop=(i == nblk - 1))
                    # normalize y on the producer, then ship 256-col halves.
                    # recip on the denominator row (partition 64), bounce it
                    # through DRAM, partition-broadcast with GpSimd, multiply.
                    a2a_in = a2a_in1 if b == 0 else a2a_in2
                    rr = _t(pool_row, [VW, HPC, 512], F32, "rr")
                    for h in range(HPC):
                        nc.vector.tensor_copy(rr[D:VW, h, :], psy[h][D:VW, :])
                        nc.sync.dma_start(den_d[g, h], rr[D:VW, h, :])
                    d0 = 2 * (g % 4)
                    for h in range(HPC):
                        rbc = _t(pool_bc, [64, 512], F32, "rbc")
                        nc.gpsimd.dma_start(rbc[:], _pbc(den_d[g, h], 64))
                        rec = _t(pool_bc, [64, 512], F32, "rec")
                        nc.vector.reciprocal_approx_fast(rec[:], rbc[:])
                        yt = _t(pool_yt, [64, 512], BF16, "yt")
                        nc.vector.tensor_tensor(yt[:], psy[h][0:D, :],
                                                rec[:], OP.mult)
                        nc.sync.dma_start(a2a_in[d0, h].opt(), yt[:, 0:HALF])
                        nc.sync.dma_start(a2a_in[d0 + 1, h].opt(),
                                          yt[:, HALF:512])

                # --- batch 0 (stats/squares pipelined ahead of qkv) ---
                squares(0)
                stats_chunk(0)
                squares(1)
                for j in range(4):
                    qkv_chunk(j)
                    stats_chunk(j + 1)
                    squares(j + 2)
                    att_chunk(0, j)
                # abc(5) must precede the A2A on the gpsimd queue
                stats_chunk(5)
                squares(6)
                nc.gpsimd.collective_compute(
                    "AllToAll", OP.bypass,
                    replica_groups=[list(range(N_CORES))],
                    ins=[a2a_in1.opt()], outs=[a2a_out1.opt()],
                )
                # --- batch 1 ---
                for j in range(4):
                    qkv_chunk(j + 4)
                    if j + 6 < NCHUNK:
                        stats_chunk(j + 6)
                    if j + 7 < NCHUNK:
                        squares(j + 7)
                    att_chunk(1, j)
                nc.gpsimd.collective_compute(
                    "AllToAll", OP.bypass,
                    replica_groups=[list(range(N_CORES))],
                    ins=[a2a_in2.opt()], outs=[a2a_out2.opt()],
                )

            # ---------- Phase D: AO proj + LN2 + MLP on 256-token halves ----
            with ExitStack() as es3:
                psD = es3.enter_context(tc.tile_pool(name="psD", bufs=5,
                                                     space="PSUM"))
                pool_x3 = es3.enter_context(tc.tile_pool(name="x3", bufs=1))
                pool_ya = es3.enter_context(tc.tile_pool(name="ya", bufs=8))
                pool_ao = es3.enter_context(tc.tile_pool(name="ao", bufs=8))
                pool_aob = es3.enter_context(tc.tile_pool(name="aob", bufs=8))
                pool_sq = es3.enter_context(tc.tile_pool(name="sq", bufs=3))
                pool_h2 = es3.enter_context(tc.tile_pool(name="h2", bufs=8))
                pool_mt = es3.enter_context(tc.tile_pool(name="mt", bufs=NFB))
                pool_wa = es3.enter_context(tc.tile_pool(name="wa", bufs=8))
                pool_w3 = es3.enter_context(tc.tile_pool(name="w3", bufs=3))
                pool_wm = es3.enter_context(tc.tile_pool(name="wm", bufs=3))
                pool_row2 = es3.enter_context(tc.tile_pool(name="row2", bufs=2))
                pool_bc2 = es3.enter_context(tc.tile_pool(name="bc2", bufs=2))
                pool_tmp2 = es3.enter_context(tc.tile_pool(name="tmp2", bufs=2))
                pool_ot = es3.enter_context(tc.tile_pool(name="ot", bufs=2))

                xts = _t(pool_x3, [128, NCH, 2 * HALF], F32, "xts")
                nc.sync.dma_start(
                    xts[:], xTs_d.ap().rearrange("(kb p) t -> p kb t", p=128))
                bao_t = _t(singles, [128, NCH], F32, "bao")
                nc.sync.dma_start(bao_t[:],
                                  bao_d.ap().rearrange("(o p) -> p o", p=128))
                bfc_t = _t(singles, [128, NFB], F32, "bfc")
                nc.sync.dma_start(bfc_t[:],
                                  bfc_d.ap().rearrange("(o p) -> p o", p=128))
                bmp_t = _t(singles, [128, NCH], F32, "bmp")
                nc.sync.dma_start(bmp_t[:],
                                  bmp_d.ap().rearrange("(o p) -> p o", p=128))
                # AO weights resident across both halves (2 MB)
                wao_t = []
                for w in range(NCH):
                    wt = _t(pool_wa, [128, NCH, 128], BF16, "wa")
                    nc.sync.dma_start(
                        wt[:],
                        wao_d.ap()[:, 128 * w:128 * (w + 1)].rearrange(
                            "(kb p) m -> p kb m", p=128))
                    wao_t.append(wt)

                def phase_d_half(half, a2a_out):
                    c0 = HALF * half
                    yall = []
                    for i in range(N_CORES):
                        ya = _t(pool_ya, [128, HALF], BF16, "ya")
                        nc.sync.dma_start(ya[:], a2a_out[i].opt())
                        yall.append(ya)
                    aot, aob = [], []
                    for w in range(NCH):
                        ps = _t(psD, [128, HALF], F32, "psD")
                        for i in range(N_CORES):
                            nc.tensor.matmul(ps[:], wao_t[w][:, i, :],
                                             yall[i][:],
                                             start=(i == 0),
                                             stop=(i == N_CORES - 1))
                        ao = _t(pool_ao, [128, HALF], F32, "ao")
                        nc.vector.scalar_tensor_tensor(
                            out=ao[:], in0=ps[:], scalar=bao_t[:, w:w + 1],
                            in1=xts[:, w, c0:c0 + HALF], op0=OP.add, op1=OP.add)
                        ab = _t(pool_aob, [128, HALF], BF16, "aob")
                        nc.vector.tensor_copy(ab[:], ao[:])
                        aot.append(ao)
                        aob.append(ab)
                    # LN2 stats: s1 row 0, s2 row 32, single PSUM bank
                    sp = _t(psD, [64, HALF], F32, "psD")
                    sqs = []
                    for w in range(NCH):
                        s = _t(pool_sq, [128, HALF], BF16, "sq")
                        nc.vector.tensor_tensor(s[:], aob[w][:], aob[w][:],
                                                OP.mult)
                        sqs.append(s)
                    for w in range(NCH):
                        nc.tensor.matmul(sp[0:1, :], ones_t[:], aob[w][:],
                                         start=(w == 0), stop=(w == NCH - 1))
                    for w in range(NCH):
                        nc.tensor.matmul(sp[32:33, :], ones_t[:], sqs[w][:],
                                         start=False, stop=(w == NCH - 1),
                                         skip_group_check=True)
                    mur = _t(pool_row2, [1, HALF], F32, "mur")
                    nc.vector.tensor_scalar(out=mur[:], in0=sp[0:1, :],
                                            scalar1=1.0 / C, scalar2=0.0,
                                            op0=OP.mult, op1=OP.add)
                    s1sq = _t(pool_row2, [1, HALF], F32, "s1sq2")
                    nc.scalar.activation(s1sq[:], sp[0:1, :], AF.Square)
                    varp = _t(pool_row2, [1, HALF], F32, "varp2")
                    nc.vector.scalar_tensor_tensor(
                        out=varp[:], in0=s1sq[:], scalar=-1.0 / C,
                        in1=sp[32:33, :], op0=OP.mult, op1=OP.add)
                    lnv = _t(pool_row2, [1, HALF], F32, "lnv2")
                    nc.scalar.activation(lnv[:], varp[:], AF.Ln,
                                         bias=eps_t[0:1, :], scale=1.0 / C)
                    rrow = _t(pool_row2, [1, HALF], F32, "rrow2")
                    nc.scalar.activation(rrow[:], lnv[:], AF.Exp, scale=-0.5)
                    mup = _t(psD, [128, HALF], F32, "psD")
                    nc.tensor.matmul(mup[:], onesrow32_t[:], mur[:])
                    mubc = _t(pool_bc2, [128, HALF], F32, "mubc")
                    nc.vector.tensor_copy(mubc[:], mup[:])
                    rbp = _t(psD, [128, HALF], F32, "psD")
                    nc.tensor.matmul(rbp[:], onesrow32_t[:], rrow[:])
                    rbc2 = _t(pool_bc2, [128, HALF], F32, "rbc2")
                    nc.vector.tensor_copy(rbc2[:], rbp[:])
                    h2 = []
                    for w in range(NCH):
                        tp = _t(pool_tmp2, [128, HALF], F32, "tmp2")
                        nc.vector.tensor_tensor(tp[:], aot[w][:], mubc[:],
                                                OP.subtract)
                        ht = _t(pool_h2, [128, HALF], BF16, "h2")
                        nc.vector.tensor_tensor(ht[:], tp[:], rbc2[:], OP.mult)
                        h2.append(ht)
                    # FC + GELU
                    mt = []
                    for fg in range(NFB // 4):
                        wt = _t(pool_w3, [128, NCH, 4, 128], BF16, "w3")
                        nc.sync.dma_start(
                            wt[:],
                            wfc_d.ap()[:, 512 * fg:512 * (fg + 1)].rearrange(
                                "(kb p) (fs m) -> p kb fs m", p=128, m=128))
                        for fs in range(4):
                            f = 4 * fg + fs
                            ps = _t(psD, [128, HALF], F32, "psD")
                            for cb in range(NCH):
                                nc.tensor.matmul(ps[:], wt[:, cb, fs, :],
                                                 h2[cb][:],
                                                 start=(cb == 0),
                                                 stop=(cb == NCH - 1))
                            m = _t(pool_mt, [128, HALF], BF16, "mt")
                            nc.scalar.activation(m[:], ps[:], AF.Gelu,
                                                 bias=bfc_t[:, f:f + 1],
                                                 scale=1.0)
                            mt.append(m)
                    # MP + bias + residual -> out
                    for w in range(NCH):
                        wt = _t(pool_wm, [128, NFB, 128], BF16, "wm")
                        nc.sync.dma_start(
                            wt[:],
                            wmp_d.ap().rearrange("(fb p) o -> p fb o", p=128)[
                                :, :, 128 * w:128 * (w + 1)])
                        ps = _t(psD, [128, HALF], F32, "psD")
                        for f in range(NFB):
                            nc.tensor.matmul(ps[:], wt[:, f, :], mt[f][:],
                                             start=(f == 0),
                                             stop=(f == NFB - 1))
                        ot = _t(pool_ot, [128, HALF], F32, "ot")
                        nc.vector.scalar_tensor_tensor(
                            out=ot[:], in0=ps[:], scalar=bmp_t[:, w:w + 1],
                            in1=aot[w][:], op0=OP.add, op1=OP.add)
                        nc.sync.dma_start(
                            out_d[128 * w:128 * (w + 1), c0:c0 + HALF], ot[:])

                phase_d_half(0, a2a_out1)
                phase_d_half(1, a2a_out2)

    nc.compile()
    return nc


def _prep(inputs):
    """Host-side preprocessing: fold LN affines into weights, slice per core."""
    f32 = np.float32
    bf16 = ml_dtypes.bfloat16
    x = np.asarray(inputs["x"], f32).reshape(NTOK, C)
    W_qkv = np.asarray(inputs["W_qkv"], f32)
    b_qkv = np.asarray(inputs["b_qkv"], f32)
    W_ao = np.asarray(inputs["W_ao"], f32)
    b_ao = np.asarray(inputs["b_ao"], f32)
    W_fc = np.asarray(inputs["W_fc"], f32)
    b_fc = np.asarray(inputs["b_fc"], f32)
    W_mp = np.asarray(inputs["W_mp"], f32)
    b_mp = np.asarray(inputs["b_mp"], f32)
    g1 = np.asarray(inputs["g1"], f32)
    be1 = np.asarray(inputs["be1"], f32)
    g2 = np.asarray(inputs["g2"], f32)
    be2 = np.asarray(inputs["be2"], f32)

    Wq_eff = W_qkv * g1[:, None]
    bq_eff = b_qkv + be1 @ W_qkv
    # fold 1/sqrt(D) into the Q columns
    Wq_eff[:, :C] *= 1.0 / np.sqrt(D)
    bq_eff[:C] *= 1.0 / np.sqrt(D)
    Wfc_eff = W_fc * g2[:, None]
    bfc_eff = b_fc + be2 @ W_fc

    xT = np.ascontiguousarray(x.T)
    xT_bf = xT.astype(bf16)
    mask = (np.arange(128)[:, None] <= np.arange(128)[None, :]).astype(bf16)
    ident = np.eye(128, dtype=bf16)

    wao_bf = W_ao.astype(bf16)
    wfc_bf = Wfc_eff.astype(bf16)
    wmp_bf = W_mp.astype(bf16)

    with_bias_qkv = bool(np.any(bq_eff != 0.0))

    in_maps = []
    for r in range(N_CORES):
        cs = 128 * r
        wq_core = np.concatenate(
            [Wq_eff[:, cs:cs + 128], Wq_eff[:, C + cs:C + cs + 128],
             Wq_eff[:, 2 * C + cs:2 * C + cs + 128]], axis=1)
        bq_core = np.concatenate(
            [bq_eff[cs:cs + 128], bq_eff[C + cs:C + cs + 128],
             bq_eff[2 * C + cs:2 * C + cs + 128]])
        wsum_core = wq_core.sum(axis=0).astype(f32)
        # phase-D half-shards: 256 tokens of batch 0 + 256 of batch 1
        xts_core = np.concatenate(
            [xT[:, HALF * r:HALF * (r + 1)],
             xT[:, T + HALF * r:T + HALF * (r + 1)]], axis=1)
        in_maps.append({
            "xT": xT_bf,
            "xTs": np.ascontiguousarray(xts_core),
            "wqkv": wq_core.astype(bf16),
            "wsum": np.ascontiguousarray(wsum_core).astype(bf16).reshape(1, -1),
            "bqkv": np.ascontiguousarray(bq_core).astype(bf16).reshape(1, -1),
            "wao": wao_bf,
            "bao": b_ao,
            "wfc": wfc_bf,
            "bfc": bfc_eff.astype(f32),
            "wmp": wmp_bf,
            "bmp": b_mp,
            "mask": mask,
            "ident": ident,
        })
    return in_maps, with_bias_qkv


def kernel(_trace=False, _trace_kwargs=None, **inputs):
    in_maps, with_bias_qkv = _prep(inputs)
    key = ("nc", with_bias_qkv)
    if key not in _CACHE:
        _CACHE[key] = _build(with_bias_qkv)
    nc = _CACHE[key]
    res = run_bass_kernel_spmd(
        nc, in_maps, core_ids=list(range(N_CORES)),
        trace=_trace, **(_trace_kwargs or {}))
    _CACHE["last_results"] = res
    # core r output: cols 0-255 = batch-0 tokens [256r,256r+256),
    #                cols 256-511 = batch-1 tokens [256r,256r+256)
    out = np.empty((B, T, C), np.float32)
    for r in range(N_CORES):
        o = np.asarray(res.results[r]["out"])
        out[0, HALF * r:HALF * (r + 1)] = o[:, 0:HALF].T
        out[1, HALF * r:HALF * (r + 1)] = o[:, HALF:2 * HALF].T
    return out
